# revision 34
# baseline (speedup 1.0000x reference)
"""Bass/Tile Trainium2 kernel for masked-bank BatchConv2D.

Math (matches the reference nn.Module):
    mask[o, j]   = j < connect_nums[o]                       (j in [0, 64))
    kdense[o, c] = sum_{j : j%32==c} weights[o, j] * mask[o, j]   -> [64, 32, 3, 3]
    out          = conv2d(x, kdense, VALID) + bias[None]          -> [B, 64, 126, 126]

Strategy: data-parallel over batch (8 cores x 4 images). Per core, conv is
computed as 3 accumulating matmuls (one per kernel-column dx) with the
contraction dim packed as (dy, c) = 96 partitions. Input-image chunks are
replicated on-chip into 3 row-shifted partition blocks (X3 tile, built by
one HBM load + two SBUF->SBUF shift DMAs).

The kernel is SDMA-engine bound (16 engines/core, ~27 GB/s each), so the
main loop minimizes bytes through the DMA fabric:
  - operands and PE stream in bf16 (PE column-pair tiling: psum lower half
    = image A, upper half = image B, same output rows -> both halves share
    identical bias rows and stores are fully contiguous per partition)
  - bias is read from HBM once ([64, HO*WO] f32) and duplicated to
    partitions 64:128 by an on-chip SBUF->SBUF copy (fabric, not HBM)
  - outputs are stored as bf16 (the rounding happens on-device in the DVE
    psum-evacuation add; the host only widens bf16->f32, which is exact)
  - stores are one DMA per (image, 32-row chunk): [64 part, 8 KB contig]

Modes (BASS_CONV_MODE): "bf16" (default) as above; "f32" exact fp32 with
the same image-paired layout (f32 stores); "f32r" streams fp32 through the
PE's single-pass FP32R mode (~tf32, rel err ~2e-4) with a serial 64-part
psum layout (ISA: f32r matmul dst must start at partition 0, so no column
pairing).
"""

import os
import sys

for _p in ("/opt/trn_rl_repo",):
    if os.path.isdir(_p) and _p not in sys.path:
        sys.path.append(_p)

import numpy as np

# Problem dims (hardcoded per contract)
B, CIN, COUT = 32, 32, 64
H, W = 128, 128
KH = KW = 3
HO = WO = 126
MAXCN = 64
NCORES = 8
BL = B // NCORES  # local batch per core

# chunks of output rows per image: (x_row_start, n_x_rows, out_row_start, n_out_rows)
CHUNKS = [(0, 34, 0, 32), (32, 34, 32, 32), (64, 34, 64, 32), (96, 32, 96, 30)]
X3W = 34 * W  # x3 tile free size (elements)

_MODE = os.environ.get("BASS_CONV_MODE", "bf16")

_RUNNER_CACHE = {}


def _split_waits(nc, mybir, maxw=1):
    """This walrus build only accepts one sem-wait per instruction; hoist
    extra waits onto preceding NoOps on the same engine."""
    for f in nc.m.functions:
        for bb in f.blocks:
            newlist = []
            for inst in bb.instructions:
                si = inst.sync_info
                waits = list(si.on_wait) if si and si.on_wait else []
                if len(waits) > maxw:
                    chunks = [waits[i : i + maxw] for i in range(0, len(waits), maxw)]
                    for ci, ch in enumerate(chunks[:-1]):
                        nop = mybir.InstNoOp(
                            name=f"{inst.name}-ws{ci}", ins=[], outs=[]
                        )
                        nop.engine = inst.engine
                        nop.sync_info = mybir.SyncInfo(on_wait=list(ch), on_update=[])
                        newlist.append(nop)
                    si.on_wait = chunks[-1]
                newlist.append(inst)
            bb.instructions = newlist


def build_nc(mode=_MODE, split_waits=True):
    import concourse.bass as bass
    import concourse.mybir as mybir
    from concourse.tile import TileContext

    f32 = mybir.dt.float32
    i32 = mybir.dt.int32
    if mode == "bf16":
        mmdt = mybir.dt.bfloat16
    elif mode == "f32r":
        mmdt = mybir.dt.float32r
    else:
        mmdt = f32
    # storage dtype of matmul operand tiles: the BIR verifier requires fp32r
    # matmul operands to be *produced* as float32r, so the x3/lhsT tiles are
    # declared float32r and the copies into them perform the rounding.
    stdt = mmdt if mode in ("bf16", "f32r") else f32
    # DRAM output dtype: bf16 mode stores rounded outputs (host widens).
    outdt = mybir.dt.bfloat16 if mode == "bf16" else f32

    # f32r matmuls cannot target psum partitions 64:128 (ISA: dst partition
    # must be 0 for 4-byte non-exact modes), so f32r runs the "serial"
    # layout: one [64, N] psum tile at base 0 per output row-tile. bf16/f32
    # run the "paired" layout: two images concurrently via PE column
    # tiling (psum halves 0:64 / 64:128, same output rows).
    paired = mode != "f32r"

    nc = bass.Bass()
    xs = nc.declare_dram_parameter("xs", [BL, CIN, H, W], f32, isOutput=False)
    wt = nc.declare_dram_parameter("wt", [COUT, MAXCN * 9], f32, isOutput=False)
    bias = nc.declare_dram_parameter("bias", [COUT, HO * WO], f32, isOutput=False)
    cn = nc.declare_dram_parameter("cn", [COUT, 1], i32, isOutput=False)
    # iota9[o, j*9+k] = j (slot index replicated over the 9 kernel taps)
    iota9 = nc.declare_dram_parameter("iota9", [COUT, MAXCN * 9], f32, isOutput=False)
    ident = nc.declare_dram_parameter("ident", [COUT, COUT], f32, isOutput=False)
    out = nc.declare_dram_parameter("out", [BL, COUT, HO * WO], outdt, isOutput=True)

    # queue-mode SBUF allocator: freed prep-pool space is not immediately
    # reused by the x3 pool, so x3 loads don't inherit a WAR dependency on
    # the weight-prep chain
    # bias is kept on-chip in bf16 in bf16 mode (bias magnitude is ~0.1 of
    # the conv output, so its rounding is far below the output's own bf16
    # rounding); halves the bias bytes through the DMA fabric.
    biasdt = mybir.dt.bfloat16 if mode == "bf16" else f32

    with TileContext(nc, pool_alloc_mode="queue") as tc:
        with tc.tile_pool(name="const", bufs=1) as constp:
            # persistent tiles
            bias2 = constp.tile([128 if paired else 64, HO * WO], biasdt)
            lhsT = constp.tile([96, 3 * COUT], stdt)
            # K=32 weight bank for the "direct" msteps (no x3 replication):
            # col block (dy*3+dx) holds that tap's [32c, 64o] weights
            lhsT32 = (
                constp.tile([32, 9 * COUT], stdt, name="lhsT32")
                if paired
                else None
            )

            # ---- weight prep ----
            with (
                tc.tile_pool(name="prep", bufs=1) as prepp,
                tc.tile_pool(name="tps", bufs=3, space="PSUM") as tpsp,
            ):
                # consts ride the scalar (Q10) queue: it is empty at startup
                # while gpsimd/sync carry the x3 prefetch flood, so these
                # tiny transfers complete quickly and unblock weight prep.
                prep_dmas = []
                cn_i = prepp.tile([COUT, 1], i32)
                prep_dmas.append(nc.scalar.dma_start(out=cn_i[:], in_=cn[:]))
                w_sb = prepp.tile([COUT, MAXCN * 9], f32)
                prep_dmas.append(nc.scalar.dma_start(out=w_sb[:], in_=wt[:]))
                iota_sb = prepp.tile([COUT, MAXCN * 9], f32)
                prep_dmas.append(nc.scalar.dma_start(out=iota_sb[:], in_=iota9[:]))
                ident_sb = prepp.tile([COUT, COUT], f32)
                prep_dmas.append(nc.scalar.dma_start(out=ident_sb[:], in_=ident[:]))

                cn_f = prepp.tile([COUT, 1], f32)
                nc.vector.tensor_copy(out=cn_f[:], in_=cn_i[:])
                # mask9[o, j*9+k] = (j < cn[o]) -> 1.0 / 0.0
                mask9 = prepp.tile([COUT, MAXCN * 9], f32)
                nc.vector.tensor_scalar(
                    out=mask9[:],
                    in0=iota_sb[:],
                    scalar1=cn_f[:],
                    scalar2=None,
                    op0=mybir.AluOpType.is_lt,
                )
                wm = prepp.tile([COUT, MAXCN * 9], f32)
                nc.vector.tensor_mul(out=wm[:], in0=w_sb[:], in1=mask9[:])
                # fold j and j+32 (same input channel): kd[o, (c, dy, dx)]
                kd = prepp.tile([COUT, CIN * 9], f32)
                nc.vector.tensor_add(
                    out=kd[:], in0=wm[:, 0 : CIN * 9], in1=wm[:, CIN * 9 : MAXCN * 9]
                )
                # reorder to (dx, dy, c) contiguous, then transpose per dx:
                # [64, (dy, c)] -> [96, 64]
                kd4 = kd.rearrange("p (c dy dx) -> p dx dy c", c=CIN, dy=3, dx=3)
                kdr = prepp.tile([COUT, CIN * 9], f32)
                kdr4 = kdr.rearrange("p (dx dy c) -> p dx dy c", c=CIN, dy=3, dx=3)
                # paired mode stores contraction rows as (dy0, dy2, dy1): the
                # x3 shift chain puts shift+1 in partitions 64:96 (so the
                # intermediate odd-port block is built first) and shift+2 in
                # 32:64; serial f32r keeps natural order.
                dyslots = (0, 2, 1) if paired else (0, 1, 2)
                for dx in range(3):
                    for dyslot, dy in enumerate(dyslots):
                        nc.vector.tensor_copy(
                            out=kdr4[:, dx, dyslot], in_=kd4[:, dx, dy]
                        )
                for dx in range(3):
                    tp = tpsp.tile([96, COUT], f32)
                    nc.tensor.transpose(
                        out=tp[:],
                        in_=kdr[:, dx * 96 : (dx + 1) * 96],
                        identity=ident_sb[:],
                    )
                    nc.vector.tensor_copy(
                        out=lhsT[:, dx * COUT : (dx + 1) * COUT], in_=tp[:]
                    )
                if False:
                    # per-tap transposes land on partitions 0:32 so the
                    # K=32 weight bank needs no cross-partition copy
                    for dx in range(3):
                        for dyslot, dy in enumerate(dyslots):
                            tp32 = tpsp.tile(
                                [32, COUT],
                                f32,
                                tag="tp32",
                                name=f"tp32_{dx}_{dy}",
                            )
                            nc.tensor.transpose(
                                out=tp32[:],
                                in_=kdr4[:, dx, dyslot],
                                identity=ident_sb[:],
                            )
                            blk = dy * 3 + dx
                            nc.vector.tensor_copy(
                                out=lhsT32[
                                    :, blk * COUT : (blk + 1) * COUT
                                ],
                                in_=tp32[:],
                            )

            if paired:
                # ---- main loop: image-paired column tiling ----
                # macro-steps: (image pair, row chunk); psum lower half is
                # image 2*pr, upper half image 2*pr+1, same output rows.
                msteps = [
                    (2 * pr, r0, nxr, oy0, nor)
                    for pr in range(BL // 2)
                    for (r0, nxr, oy0, nor) in CHUNKS
                ]
                with (
                    tc.tile_pool(name="x3", bufs=6) as x3p,
                    tc.tile_pool(name="ps", bufs=8, space="PSUM") as psp,
                    tc.tile_pool(name="ob", bufs=3) as obp,
                ):
                    x3_tiles = {}

                    # SBUF AXI port map: partitions 0:64 ride the even DMA
                    # ports, 64:128 the odd ones. The x3 tile is pinned to
                    # partitions 0:96 (K=96 matmul => PE row-tile position
                    # 0), so the shift chain is routed to balance the port
                    # groups: base load writes even, shift+2 writes odd,
                    # and block1 is built FROM block2 (read odd, write even)
                    # instead of from the base block (read+write even).
                    # partition blocks hold (dy=0, dy=2, dy=1): base load
                    # writes even ports, then block2 (partitions 64:96,
                    # odd ports) takes shift+1 with one extra row, and
                    # block1 (32:64, even) takes shift+2 sourced FROM
                    # block2 (read odd, write even). This balances the
                    # SBUF AXI port groups with no fixup DMA. The first
                    # prefetched msteps instead shift both blocks straight
                    # from the base block (shorter dependency chain while
                    # the pipeline fills).
                    # msteps whose matmuls read the base block directly
                    # (9 accumulating K=32 matmuls per tile, no shift DMAs):
                    # the first mstep (compute can start as soon as the
                    # base load lands, before any shift chain) and the last
                    # (no shift chain in the drain tail).
                    k32_msteps = set()

                    def issue_x3(mi, fill=False):
                        biA, r0, nxr, oy0, nor = msteps[mi]
                        pair = []
                        for k in (0, 1):
                            x3 = x3p.tile(
                                [96, X3W], stdt, tag=f"x3{k}", name=f"x3_{mi}_{k}"
                            )
                            if mode == "bf16":
                                # dtype-cast load must ride SWDGE (gpsimd)
                                nc.gpsimd.dma_start(
                                    out=x3[0:32, 0 : nxr * W],
                                    in_=xs[biA + k, :, r0 : r0 + nxr, :],
                                )
                            else:
                                nc.sync.dma_start(
                                    out=x3[0:32, 0 : nxr * W],
                                    in_=xs[biA + k, :, r0 : r0 + nxr, :],
                                )
                            if mi in k32_msteps:
                                pair.append(x3)
                                continue
                            nc.sync.dma_start(
                                out=x3[64:96, 0 : (nor + 1) * W],
                                in_=x3[0:32, W : (nor + 2) * W],
                            )
                            if fill:
                                nc.scalar.dma_start(
                                    out=x3[32:64, 0 : nor * W],
                                    in_=x3[0:32, 2 * W : (nor + 2) * W],
                                )
                            else:
                                nc.scalar.dma_start(
                                    out=x3[32:64, 0 : nor * W],
                                    in_=x3[64:96, W : (nor + 1) * W],
                                )
                            pair.append(x3)
                        x3_tiles[mi] = pair

                    issue_x3(0)
                    issue_x3(1, fill=True)
                    issue_x3(2, fill=True)

                    # bias is loaded piecewise during the first image-pair's
                    # chunks so its 4 MB doesn't flood the DMA fabric at
                    # startup. Upper half (partitions 64:128) is an on-chip
                    # duplicate of the lower half (both psum halves cover the
                    # same output rows), built by SBUF->SBUF fabric DMA.
                    # both bias halves load straight from HBM (the upper
                    # half is the same data for partitions 64:128 -- the
                    # image-B psum half covers the same output rows); a
                    # second HBM read costs nothing on the SBUF AXI ports
                    # where the kernel is bound.
                    def issue_bias(ci):
                        _, _, oy0, nor = CHUNKS[ci]
                        a, b = oy0 * WO, (oy0 + nor) * WO
                        if mode == "bf16" and os.environ.get("BASS_BIAS_DUP") == "1":
                            nc.gpsimd.dma_start(
                                out=bias2[0:64, a:b], in_=bias[:, a:b]
                            )
                            nc.sync.dma_start(
                                out=bias2[64:128, a:b], in_=bias2[0:64, a:b]
                            )
                        elif mode == "bf16":
                            nc.gpsimd.dma_start(
                                out=bias2[0:64, a:b], in_=bias[:, a:b]
                            )
                            nc.gpsimd.dma_start(
                                out=bias2[64:128, a:b], in_=bias[:, a:b]
                            )
                        else:
                            nc.sync.dma_start(
                                out=bias2[0:64, a:b], in_=bias[:, a:b]
                            )
                            nc.sync.dma_start(
                                out=bias2[64:128, a:b], in_=bias2[0:64, a:b]
                            )

                    for mi, (biA, r0, nxr, oy0, nor) in enumerate(msteps):
                        if mi < len(CHUNKS):
                            issue_bias(mi)
                        if mi + 3 < len(msteps):
                            issue_x3(mi + 3)
                        x3A, x3B = x3_tiles.pop(mi)
                        xvA = x3A.rearrange("p (r c) -> p r c", c=W)
                        xvB = x3B.rearrange("p (r c) -> p r c", c=W)

                        # row-tiles of <=4 output rows; same tile index on
                        # both psum halves (image A lower, image B upper)
                        ntiles = (nor + 3) // 4
                        ob = obp.tile([128, 8 * 4 * WO], outdt, tag="ob")
                        t0 = 0
                        while t0 < ntiles:
                            g = min(4, ntiles - t0)
                            tss = []
                            for ti in range(t0, t0 + g):
                                nr = min(4, nor - 4 * ti)
                                ps = psp.tile([128, 4 * WO], f32, tag="ps")
                                tss.append((ti, nr, ps))
                            if mi in k32_msteps:
                                # direct path: base block only, tap (dy,dx)
                                # as 9 accumulating K=32 matmuls
                                for dy in range(3):
                                    for dx in range(3):
                                        blk = dy * 3 + dx
                                        lw = lhsT32[
                                            :, blk * COUT : (blk + 1) * COUT
                                        ]
                                        st = dy == 0 and dx == 0
                                        sp = dy == 2 and dx == 2
                                        for (ti, nr, ps) in tss:
                                            yl = 4 * ti + dy
                                            N = nr * WO
                                            nc.tensor.matmul(
                                                ps[0:64, 0:N],
                                                lhsT=lw,
                                                rhs=xvA[
                                                    0:32,
                                                    yl : yl + nr,
                                                    dx : dx + WO,
                                                ],
                                                start=st,
                                                stop=sp,
                                                skip_group_check=True,
                                            )
                                            nc.tensor.matmul(
                                                ps[64:128, 0:N],
                                                lhsT=lw,
                                                rhs=xvB[
                                                    0:32,
                                                    yl : yl + nr,
                                                    dx : dx + WO,
                                                ],
                                                start=st,
                                                stop=sp,
                                                skip_group_check=True,
                                            )
                            else:
                                for dx in range(3):
                                    lw = lhsT[:, dx * COUT : (dx + 1) * COUT]
                                    for (ti, nr, ps) in tss:
                                        yl = 4 * ti
                                        N = nr * WO
                                        nc.tensor.matmul(
                                            ps[0:64, 0:N],
                                            lhsT=lw,
                                            rhs=xvA[:, yl : yl + nr, dx : dx + WO],
                                            start=(dx == 0),
                                            stop=(dx == 2),
                                            skip_group_check=True,
                                        )
                                        nc.tensor.matmul(
                                            ps[64:128, 0:N],
                                            lhsT=lw,
                                            rhs=xvB[:, yl : yl + nr, dx : dx + WO],
                                            start=(dx == 0),
                                            stop=(dx == 2),
                                            skip_group_check=True,
                                        )
                            # fused +bias psum evacuation; both halves read
                            # the same bias rows (upper = on-chip duplicate)
                            for (ti, nr, ps) in tss:
                                yg = oy0 + 4 * ti
                                N = nr * WO
                                nc.vector.tensor_add(
                                    out=ob[:, 4 * ti * WO : 4 * ti * WO + N],
                                    in0=ps[:, 0:N],
                                    in1=bias2[:, yg * WO : yg * WO + N],
                                )
                            t0 += g
                        # one 128-partition store for both images (they are
                        # adjacent in DRAM): uses all 16 SBUF AXI ports
                        ov = out[
                            biA : biA + 2, :, oy0 * WO : (oy0 + nor) * WO
                        ].rearrange("i o n -> (i o) n")
                        nc.scalar.dma_start(out=ov, in_=ob[:, 0 : nor * WO])
            else:
                # ---- main loop: serial layout (f32r) ----
                chunks = [
                    (bi, r0, nxr, oy0, nor)
                    for bi in range(BL)
                    for (r0, nxr, oy0, nor) in CHUNKS
                ]
                with (
                    tc.tile_pool(name="x3", bufs=4) as x3p,
                    tc.tile_pool(name="ps", bufs=8, space="PSUM") as psp,
                    tc.tile_pool(name="ob", bufs=2) as obp,
                ):
                    x3_tiles = {}

                    def issue_x3s(ci):
                        bi, r0, nxr, oy0, nor = chunks[ci]
                        x3 = x3p.tile([96, X3W], stdt, tag="x3", name=f"x3_{ci}")
                        xin = xs[bi, :, r0 : r0 + nxr, :]
                        if mode == "f32r":
                            xin = xin.bitcast(mmdt)
                        nc.sync.dma_start(out=x3[0:32, 0 : nxr * W], in_=xin)
                        nc.scalar.dma_start(
                            out=x3[32:64, 0 : nor * W],
                            in_=x3[0:32, W : (nor + 1) * W],
                        )
                        nc.gpsimd.dma_start(
                            out=x3[64:96, 0 : nor * W],
                            in_=x3[0:32, 2 * W : (nor + 2) * W],
                        )
                        x3_tiles[ci] = x3

                    issue_x3s(0)
                    issue_x3s(1)

                    def issue_bias_s(ci):
                        _, _, _, oy0, nor = chunks[ci]
                        a, b = oy0 * WO, (oy0 + nor) * WO
                        nc.gpsimd.dma_start(out=bias2[0:64, a:b], in_=bias[:, a:b])

                    for ci, (bi, r0, nxr, oy0, nor) in enumerate(chunks):
                        if ci < len(CHUNKS):
                            issue_bias_s(ci)
                        if ci + 2 < len(chunks):
                            issue_x3s(ci + 2)
                        x3 = x3_tiles.pop(ci)
                        x3v = x3.rearrange("p (r c) -> p r c", c=W)
                        # serial layout: 4-row tiles, dx-outer over groups
                        # of 8 tiles (weights change every 8 matmuls)
                        ntiles = (nor + 3) // 4
                        t0 = 0
                        while t0 < ntiles:
                            g = min(8, ntiles - t0)
                            tss = []
                            for ti in range(t0, t0 + g):
                                nr = min(4, nor - 4 * ti)
                                ps = psp.tile([64, 4 * WO], f32, tag="ps")
                                tss.append((ti, nr, ps))
                            for dx in range(3):
                                lw = lhsT[:, dx * COUT : (dx + 1) * COUT]
                                for (ti, nr, ps) in tss:
                                    yl = 4 * ti
                                    nc.tensor.matmul(
                                        ps[:, 0 : nr * WO],
                                        lhsT=lw,
                                        rhs=x3v[:, yl : yl + nr, dx : dx + WO],
                                        start=(dx == 0),
                                        stop=(dx == 2),
                                        skip_group_check=True,
                                    )
                            ob = obp.tile([64, 8 * 4 * WO], f32, tag="ob")
                            rows = 0
                            for gi, (ti, nr, ps) in enumerate(tss):
                                yg = oy0 + 4 * ti
                                N = nr * WO
                                o0 = gi * 4 * WO
                                nc.vector.tensor_add(
                                    out=ob[:, o0 : o0 + N],
                                    in0=ps[:, 0:N],
                                    in1=bias2[0:64, yg * WO : yg * WO + N],
                                )
                                rows += nr
                            yg0 = oy0 + 4 * t0
                            if rows == 4 * g:
                                nc.scalar.dma_start(
                                    out=out[bi, :, yg0 * WO : (yg0 + rows) * WO],
                                    in_=ob[:, 0 : rows * WO],
                                )
                            else:
                                # ragged tail: last tile shorter; store
                                # full tiles in one DMA, tail separately
                                nf = rows - tss[-1][1]
                                nc.scalar.dma_start(
                                    out=out[bi, :, yg0 * WO : (yg0 + nf) * WO],
                                    in_=ob[:, 0 : nf * WO],
                                )
                                nc.scalar.dma_start(
                                    out=out[
                                        bi,
                                        :,
                                        (yg0 + nf) * WO : (yg0 + rows) * WO,
                                    ],
                                    in_=ob[
                                        :,
                                        (g - 1) * 4 * WO : (g - 1) * 4 * WO
                                        + tss[-1][1] * WO,
                                    ],
                                )
                            t0 += g

    if split_waits:
        _split_waits(nc, mybir)
    return nc


def _make_inputs(x, weights, bias, connect_nums):
    """Host-side reshapes only (no input-dependent compute)."""
    x = np.ascontiguousarray(np.asarray(x, dtype=np.float32))
    w = np.ascontiguousarray(
        np.asarray(weights, dtype=np.float32).reshape(COUT, MAXCN * 9)
    )
    b = np.ascontiguousarray(np.asarray(bias, dtype=np.float32).reshape(COUT, HO * WO))
    cnv = np.ascontiguousarray(
        np.asarray(connect_nums, dtype=np.int32).reshape(COUT, 1)
    )
    iota9 = np.ascontiguousarray(
        np.tile(
            np.repeat(np.arange(MAXCN, dtype=np.float32), 9), (COUT, 1)
        )
    )
    ident = np.eye(COUT, dtype=np.float32)
    shards = x.reshape(NCORES, BL, CIN, H, W)
    in_maps = [
        {
            "xs": shards[c],
            "wt": w,
            "bias": b,
            "cn": cnv,
            "iota9": iota9,
            "ident": ident,
        }
        for c in range(NCORES)
    ]
    return in_maps


def _get_runner(mode=_MODE):
    """Build + jit once; reuse across kernel() calls."""
    if mode in _RUNNER_CACHE:
        return _RUNNER_CACHE[mode]

    import jax
    from jax.experimental.shard_map import shard_map
    from jax.sharding import Mesh, PartitionSpec

    import concourse.mybir as mybir
    from concourse.bass2jax import (
        _bass_exec_p,
        install_neuronx_cc_hook,
        partition_id_tensor,
    )

    nc = build_nc(mode)
    install_neuronx_cc_hook()

    partition_name = nc.partition_id_tensor.name if nc.partition_id_tensor else None
    in_names = []
    out_names = []
    out_avals = []
    zero_shapes = []
    for alloc in nc.m.functions[0].allocations:
        if not isinstance(alloc, mybir.MemoryLocationSet):
            continue
        name = alloc.memorylocations[0].name
        if alloc.kind == "ExternalInput":
            if name != partition_name:
                in_names.append(name)
        elif alloc.kind == "ExternalOutput":
            out_names.append(name)
            shape = tuple(alloc.tensor_shape)
            dtype = mybir.dt.np(alloc.dtype)
            out_avals.append(jax.core.ShapedArray(shape, dtype))
            zero_shapes.append((shape, dtype))
    n_params = len(in_names)
    n_outs = len(out_names)
    all_names = in_names + out_names
    if partition_name is not None:
        all_names = all_names + [partition_name]

    def _body(*args):
        operands = list(args)
        if partition_name is not None:
            operands.append(partition_id_tensor())
        outs = _bass_exec_p.bind(
            *operands,
            out_avals=tuple(out_avals),
            in_names=tuple(all_names),
            out_names=tuple(out_names),
            lowering_input_output_aliases=(),
            sim_require_finite=True,
            sim_require_nnan=True,
            nc=nc,
        )
        return tuple(outs)

    devices = jax.devices()[:NCORES]
    mesh = Mesh(np.asarray(devices), ("core",))
    in_specs = (PartitionSpec("core"),) * (n_params + n_outs)
    out_specs = (PartitionSpec("core"),) * n_outs
    sharded = jax.jit(
        shard_map(
            _body, mesh=mesh, in_specs=in_specs, out_specs=out_specs, check_rep=False
        ),
        donate_argnums=tuple(range(n_params, n_params + n_outs)),
        keep_unused=True,
    )

    def run(in_maps):
        concat_in = [
            np.concatenate([np.asarray(in_maps[c][nm]) for c in range(NCORES)], axis=0)
            for nm in in_names
        ]
        concat_zeros = [
            np.zeros((NCORES * s[0],) + tuple(s[1:]), dt) for (s, dt) in zero_shapes
        ]
        out_arrs = sharded(*concat_in, *concat_zeros)
        # bf16 mode stores rounded outputs; widening to f32 is exact
        outv = np.asarray(out_arrs[0]).astype(np.float32)
        return outv.reshape(NCORES, BL, COUT, HO, WO)

    _RUNNER_CACHE[mode] = run
    return run


def kernel(x, weights, bias, connect_nums):
    run = _get_runner()
    in_maps = _make_inputs(x, weights, bias, connect_nums)
    outs = run(in_maps)
    return np.ascontiguousarray(outs.reshape(B, COUT, HO, WO))


# revision 36
# speedup vs baseline: 1.0403x; 1.0403x over previous
"""Bass/Tile Trainium2 kernel for masked-bank BatchConv2D.

Math (matches the reference nn.Module):
    mask[o, j]   = j < connect_nums[o]                       (j in [0, 64))
    kdense[o, c] = sum_{j : j%32==c} weights[o, j] * mask[o, j]   -> [64, 32, 3, 3]
    out          = conv2d(x, kdense, VALID) + bias[None]          -> [B, 64, 126, 126]

Strategy: data-parallel over batch (8 cores x 4 images). Per core, conv is
computed as 3 accumulating matmuls (one per kernel-column dx) with the
contraction dim packed as (dy, c) = 96 partitions. Input-image chunks are
replicated on-chip into 3 row-shifted partition blocks (X3 tile, built by
one HBM load + two SBUF->SBUF shift DMAs).

The kernel is SDMA-engine bound (16 engines/core, ~27 GB/s each), so the
main loop minimizes bytes through the DMA fabric:
  - operands and PE stream in bf16 (PE column-pair tiling: psum lower half
    = image A, upper half = image B, same output rows -> both halves share
    identical bias rows and stores are fully contiguous per partition)
  - bias is read from HBM once ([64, HO*WO] f32) and duplicated to
    partitions 64:128 by an on-chip SBUF->SBUF copy (fabric, not HBM)
  - outputs are stored as bf16 (the rounding happens on-device in the DVE
    psum-evacuation add; the host only widens bf16->f32, which is exact)
  - stores are one DMA per (image, 32-row chunk): [64 part, 8 KB contig]

Modes (BASS_CONV_MODE): "bf16" (default) as above; "f32" exact fp32 with
the same image-paired layout (f32 stores); "f32r" streams fp32 through the
PE's single-pass FP32R mode (~tf32, rel err ~2e-4) with a serial 64-part
psum layout (ISA: f32r matmul dst must start at partition 0, so no column
pairing).
"""

import os
import sys

for _p in ("/opt/trn_rl_repo",):
    if os.path.isdir(_p) and _p not in sys.path:
        sys.path.append(_p)

import numpy as np

# Problem dims (hardcoded per contract)
B, CIN, COUT = 32, 32, 64
H, W = 128, 128
KH = KW = 3
HO = WO = 126
MAXCN = 64
NCORES = 8
BL = B // NCORES  # local batch per core

# chunks of output rows per image: (x_row_start, n_x_rows, out_row_start, n_out_rows)
CHUNKS = [(0, 34, 0, 32), (32, 34, 32, 32), (64, 34, 64, 32), (96, 32, 96, 30)]
X3W = 34 * W  # x3 tile free size (elements)

_MODE = os.environ.get("BASS_CONV_MODE", "bf16")

_RUNNER_CACHE = {}


def _split_waits(nc, mybir, maxw=1):
    """This walrus build only accepts one sem-wait per instruction; hoist
    extra waits onto preceding NoOps on the same engine."""
    for f in nc.m.functions:
        for bb in f.blocks:
            newlist = []
            for inst in bb.instructions:
                si = inst.sync_info
                waits = list(si.on_wait) if si and si.on_wait else []
                if len(waits) > maxw:
                    chunks = [waits[i : i + maxw] for i in range(0, len(waits), maxw)]
                    for ci, ch in enumerate(chunks[:-1]):
                        nop = mybir.InstNoOp(
                            name=f"{inst.name}-ws{ci}", ins=[], outs=[]
                        )
                        nop.engine = inst.engine
                        nop.sync_info = mybir.SyncInfo(on_wait=list(ch), on_update=[])
                        newlist.append(nop)
                    si.on_wait = chunks[-1]
                newlist.append(inst)
            bb.instructions = newlist


def build_nc(mode=_MODE, split_waits=True):
    import concourse.bass as bass
    import concourse.mybir as mybir
    from concourse.tile import TileContext

    f32 = mybir.dt.float32
    i32 = mybir.dt.int32
    if mode == "bf16":
        mmdt = mybir.dt.bfloat16
    elif mode == "f32r":
        mmdt = mybir.dt.float32r
    else:
        mmdt = f32
    # storage dtype of matmul operand tiles: the BIR verifier requires fp32r
    # matmul operands to be *produced* as float32r, so the x3/lhsT tiles are
    # declared float32r and the copies into them perform the rounding.
    stdt = mmdt if mode in ("bf16", "f32r") else f32
    # DRAM output dtype: bf16 mode stores rounded outputs (host widens).
    outdt = mybir.dt.bfloat16 if mode == "bf16" else f32

    # f32r matmuls cannot target psum partitions 64:128 (ISA: dst partition
    # must be 0 for 4-byte non-exact modes), so f32r runs the "serial"
    # layout: one [64, N] psum tile at base 0 per output row-tile. bf16/f32
    # run the "paired" layout: two images concurrently via PE column
    # tiling (psum halves 0:64 / 64:128, same output rows).
    paired = mode != "f32r"

    nc = bass.Bass()
    xs = nc.declare_dram_parameter("xs", [BL, CIN, H, W], f32, isOutput=False)
    wt = nc.declare_dram_parameter("wt", [COUT, MAXCN * 9], f32, isOutput=False)
    bias = nc.declare_dram_parameter("bias", [COUT, HO * WO], f32, isOutput=False)
    cn = nc.declare_dram_parameter("cn", [COUT, 1], i32, isOutput=False)
    # iota9[o, j*9+k] = j (slot index replicated over the 9 kernel taps)
    iota9 = nc.declare_dram_parameter("iota9", [COUT, MAXCN * 9], f32, isOutput=False)
    ident = nc.declare_dram_parameter("ident", [COUT, COUT], f32, isOutput=False)
    out = nc.declare_dram_parameter("out", [BL, COUT, HO * WO], outdt, isOutput=True)

    # queue-mode SBUF allocator: freed prep-pool space is not immediately
    # reused by the x3 pool, so x3 loads don't inherit a WAR dependency on
    # the weight-prep chain
    # bias is kept on-chip in bf16 in bf16 mode (bias magnitude is ~0.1 of
    # the conv output, so its rounding is far below the output's own bf16
    # rounding); halves the bias bytes through the DMA fabric.
    biasdt = mybir.dt.bfloat16 if mode == "bf16" else f32

    with TileContext(nc, pool_alloc_mode="queue") as tc:
        with tc.tile_pool(name="const", bufs=1) as constp:
            # persistent tiles
            bias2 = constp.tile([128 if paired else 64, HO * WO], biasdt)
            lhsT = constp.tile([96, 3 * COUT], stdt)
            # K=32 weight bank for the "direct" msteps (no x3 replication):
            # col block (dy*3+dx) holds that tap's [32c, 64o] weights
            lhsT32 = (
                constp.tile([32, 9 * COUT], stdt, name="lhsT32")
                if paired
                else None
            )

            # ---- weight prep ----
            with (
                tc.tile_pool(name="prep", bufs=1) as prepp,
                tc.tile_pool(name="tps", bufs=3, space="PSUM") as tpsp,
            ):
                # consts ride the scalar (Q10) queue: it is empty at startup
                # while gpsimd/sync carry the x3 prefetch flood, so these
                # tiny transfers complete quickly and unblock weight prep.
                prep_dmas = []
                cn_i = prepp.tile([COUT, 1], i32)
                prep_dmas.append(nc.scalar.dma_start(out=cn_i[:], in_=cn[:]))
                w_sb = prepp.tile([COUT, MAXCN * 9], f32)
                prep_dmas.append(nc.scalar.dma_start(out=w_sb[:], in_=wt[:]))
                iota_sb = prepp.tile([COUT, MAXCN * 9], f32)
                prep_dmas.append(nc.scalar.dma_start(out=iota_sb[:], in_=iota9[:]))
                ident_sb = prepp.tile([COUT, COUT], f32)
                prep_dmas.append(nc.scalar.dma_start(out=ident_sb[:], in_=ident[:]))

                cn_f = prepp.tile([COUT, 1], f32)
                nc.vector.tensor_copy(out=cn_f[:], in_=cn_i[:])
                # mask9[o, j*9+k] = (j < cn[o]) -> 1.0 / 0.0
                mask9 = prepp.tile([COUT, MAXCN * 9], f32)
                nc.vector.tensor_scalar(
                    out=mask9[:],
                    in0=iota_sb[:],
                    scalar1=cn_f[:],
                    scalar2=None,
                    op0=mybir.AluOpType.is_lt,
                )
                wm = prepp.tile([COUT, MAXCN * 9], f32)
                nc.vector.tensor_mul(out=wm[:], in0=w_sb[:], in1=mask9[:])
                # fold j and j+32 (same input channel): kd[o, (c, dy, dx)]
                kd = prepp.tile([COUT, CIN * 9], f32)
                nc.vector.tensor_add(
                    out=kd[:], in0=wm[:, 0 : CIN * 9], in1=wm[:, CIN * 9 : MAXCN * 9]
                )
                # reorder to (dx, dy, c) contiguous, then transpose per dx:
                # [64, (dy, c)] -> [96, 64]
                kd4 = kd.rearrange("p (c dy dx) -> p dx dy c", c=CIN, dy=3, dx=3)
                kdr = prepp.tile([COUT, CIN * 9], f32)
                kdr4 = kdr.rearrange("p (dx dy c) -> p dx dy c", c=CIN, dy=3, dx=3)
                # paired mode stores contraction rows as (dy0, dy2, dy1): the
                # x3 shift chain puts shift+1 in partitions 64:96 (so the
                # intermediate odd-port block is built first) and shift+2 in
                # 32:64; serial f32r keeps natural order.
                dyslots = (0, 2, 1) if paired else (0, 1, 2)
                for dx in range(3):
                    for dyslot, dy in enumerate(dyslots):
                        nc.vector.tensor_copy(
                            out=kdr4[:, dx, dyslot], in_=kd4[:, dx, dy]
                        )
                for dx in range(3):
                    tp = tpsp.tile([96, COUT], f32)
                    nc.tensor.transpose(
                        out=tp[:],
                        in_=kdr[:, dx * 96 : (dx + 1) * 96],
                        identity=ident_sb[:],
                    )
                    nc.vector.tensor_copy(
                        out=lhsT[:, dx * COUT : (dx + 1) * COUT], in_=tp[:]
                    )
                if False:
                    # per-tap transposes land on partitions 0:32 so the
                    # K=32 weight bank needs no cross-partition copy
                    for dx in range(3):
                        for dyslot, dy in enumerate(dyslots):
                            tp32 = tpsp.tile(
                                [32, COUT],
                                f32,
                                tag="tp32",
                                name=f"tp32_{dx}_{dy}",
                            )
                            nc.tensor.transpose(
                                out=tp32[:],
                                in_=kdr4[:, dx, dyslot],
                                identity=ident_sb[:],
                            )
                            blk = dy * 3 + dx
                            nc.vector.tensor_copy(
                                out=lhsT32[
                                    :, blk * COUT : (blk + 1) * COUT
                                ],
                                in_=tp32[:],
                            )

            if paired:
                # ---- main loop: image-paired column tiling ----
                # macro-steps: (image pair, row chunk); psum lower half is
                # image 2*pr, upper half image 2*pr+1, same output rows.
                msteps = [
                    (2 * pr, r0, nxr, oy0, nor)
                    for pr in range(BL // 2)
                    for (r0, nxr, oy0, nor) in CHUNKS
                ]
                with (
                    tc.tile_pool(name="x3", bufs=6) as x3p,
                    tc.tile_pool(name="ps", bufs=8, space="PSUM") as psp,
                    tc.tile_pool(name="ob", bufs=3) as obp,
                ):
                    x3_tiles = {}

                    # SBUF AXI port map: partitions 0:64 ride the even DMA
                    # ports, 64:128 the odd ones. The x3 tile is pinned to
                    # partitions 0:96 (K=96 matmul => PE row-tile position
                    # 0), so the shift chain is routed to balance the port
                    # groups: base load writes even, shift+2 writes odd,
                    # and block1 is built FROM block2 (read odd, write even)
                    # instead of from the base block (read+write even).
                    # partition blocks hold (dy=0, dy=2, dy=1): base load
                    # writes even ports, then block2 (partitions 64:96,
                    # odd ports) takes shift+1 with one extra row, and
                    # block1 (32:64, even) takes shift+2 sourced FROM
                    # block2 (read odd, write even). This balances the
                    # SBUF AXI port groups with no fixup DMA. The first
                    # prefetched msteps instead shift both blocks straight
                    # from the base block (shorter dependency chain while
                    # the pipeline fills).
                    # msteps whose matmuls read the base block directly
                    # (9 accumulating K=32 matmuls per tile, no shift DMAs):
                    # the first mstep (compute can start as soon as the
                    # base load lands, before any shift chain) and the last
                    # (no shift chain in the drain tail).
                    k32_msteps = set()

                    def issue_x3(mi, fill=False):
                        biA, r0, nxr, oy0, nor = msteps[mi]
                        pair = []
                        for k in (0, 1):
                            x3 = x3p.tile(
                                [96, X3W], stdt, tag=f"x3{k}", name=f"x3_{mi}_{k}"
                            )
                            if mode == "bf16":
                                # dtype-cast load must ride SWDGE (gpsimd)
                                nc.gpsimd.dma_start(
                                    out=x3[0:32, 0 : nxr * W],
                                    in_=xs[biA + k, :, r0 : r0 + nxr, :],
                                )
                            else:
                                nc.sync.dma_start(
                                    out=x3[0:32, 0 : nxr * W],
                                    in_=xs[biA + k, :, r0 : r0 + nxr, :],
                                )
                            if mi in k32_msteps:
                                pair.append(x3)
                                continue
                            nc.sync.dma_start(
                                out=x3[64:96, 0 : (nor + 1) * W],
                                in_=x3[0:32, W : (nor + 2) * W],
                            )
                            if fill:
                                nc.scalar.dma_start(
                                    out=x3[32:64, 0 : nor * W],
                                    in_=x3[0:32, 2 * W : (nor + 2) * W],
                                )
                            else:
                                nc.scalar.dma_start(
                                    out=x3[32:64, 0 : nor * W],
                                    in_=x3[64:96, W : (nor + 1) * W],
                                )
                            pair.append(x3)
                        x3_tiles[mi] = pair

                    issue_x3(0)
                    issue_x3(1, fill=True)
                    issue_x3(2, fill=True)

                    # bias is loaded piecewise during the first image-pair's
                    # chunks so its 4 MB doesn't flood the DMA fabric at
                    # startup. Upper half (partitions 64:128) is an on-chip
                    # duplicate of the lower half (both psum halves cover the
                    # same output rows), built by SBUF->SBUF fabric DMA.
                    # both bias halves load straight from HBM (the upper
                    # half is the same data for partitions 64:128 -- the
                    # image-B psum half covers the same output rows); a
                    # second HBM read costs nothing on the SBUF AXI ports
                    # where the kernel is bound.
                    def issue_bias(ci):
                        _, _, oy0, nor = CHUNKS[ci]
                        a, b = oy0 * WO, (oy0 + nor) * WO
                        if mode == "bf16":
                            nc.gpsimd.dma_start(
                                out=bias2[0:64, a:b], in_=bias[:, a:b]
                            )
                            nc.sync.dma_start(
                                out=bias2[64:128, a:b], in_=bias2[0:64, a:b]
                            )
                        else:
                            nc.sync.dma_start(
                                out=bias2[0:64, a:b], in_=bias[:, a:b]
                            )
                            nc.sync.dma_start(
                                out=bias2[64:128, a:b], in_=bias2[0:64, a:b]
                            )

                    for mi, (biA, r0, nxr, oy0, nor) in enumerate(msteps):
                        if mi < len(CHUNKS):
                            issue_bias(mi)
                        if mi + 3 < len(msteps):
                            issue_x3(mi + 3)
                        x3A, x3B = x3_tiles.pop(mi)
                        xvA = x3A.rearrange("p (r c) -> p r c", c=W)
                        xvB = x3B.rearrange("p (r c) -> p r c", c=W)

                        # row-tiles of <=4 output rows; same tile index on
                        # both psum halves (image A lower, image B upper)
                        ntiles = (nor + 3) // 4
                        ob = obp.tile([128, 8 * 4 * WO], outdt, tag="ob")
                        t0 = 0
                        GSZ = int(os.environ.get("BASS_GSZ", "4"))
                        while t0 < ntiles:
                            g = min(GSZ, ntiles - t0)
                            tss = []
                            for ti in range(t0, t0 + g):
                                nr = min(4, nor - 4 * ti)
                                ps = psp.tile([128, 4 * WO], f32, tag="ps")
                                tss.append((ti, nr, ps))
                            if mi in k32_msteps:
                                # direct path: base block only, tap (dy,dx)
                                # as 9 accumulating K=32 matmuls
                                for dy in range(3):
                                    for dx in range(3):
                                        blk = dy * 3 + dx
                                        lw = lhsT32[
                                            :, blk * COUT : (blk + 1) * COUT
                                        ]
                                        st = dy == 0 and dx == 0
                                        sp = dy == 2 and dx == 2
                                        for (ti, nr, ps) in tss:
                                            yl = 4 * ti + dy
                                            N = nr * WO
                                            nc.tensor.matmul(
                                                ps[0:64, 0:N],
                                                lhsT=lw,
                                                rhs=xvA[
                                                    0:32,
                                                    yl : yl + nr,
                                                    dx : dx + WO,
                                                ],
                                                start=st,
                                                stop=sp,
                                                skip_group_check=True,
                                            )
                                            nc.tensor.matmul(
                                                ps[64:128, 0:N],
                                                lhsT=lw,
                                                rhs=xvB[
                                                    0:32,
                                                    yl : yl + nr,
                                                    dx : dx + WO,
                                                ],
                                                start=st,
                                                stop=sp,
                                                skip_group_check=True,
                                            )
                            else:
                                for dx in range(3):
                                    lw = lhsT[:, dx * COUT : (dx + 1) * COUT]
                                    for (ti, nr, ps) in tss:
                                        yl = 4 * ti
                                        N = nr * WO
                                        nc.tensor.matmul(
                                            ps[0:64, 0:N],
                                            lhsT=lw,
                                            rhs=xvA[:, yl : yl + nr, dx : dx + WO],
                                            start=(dx == 0),
                                            stop=(dx == 2),
                                            skip_group_check=True,
                                        )
                                        nc.tensor.matmul(
                                            ps[64:128, 0:N],
                                            lhsT=lw,
                                            rhs=xvB[:, yl : yl + nr, dx : dx + WO],
                                            start=(dx == 0),
                                            stop=(dx == 2),
                                            skip_group_check=True,
                                        )
                            # fused +bias psum evacuation; both halves read
                            # the same bias rows (upper = on-chip duplicate)
                            for (ti, nr, ps) in tss:
                                yg = oy0 + 4 * ti
                                N = nr * WO
                                nc.vector.tensor_add(
                                    out=ob[:, 4 * ti * WO : 4 * ti * WO + N],
                                    in0=ps[:, 0:N],
                                    in1=bias2[:, yg * WO : yg * WO + N],
                                )
                            t0 += g
                        # one 128-partition store for both images (they are
                        # adjacent in DRAM): uses all 16 SBUF AXI ports
                        ov = out[
                            biA : biA + 2, :, oy0 * WO : (oy0 + nor) * WO
                        ].rearrange("i o n -> (i o) n")
                        nc.scalar.dma_start(out=ov, in_=ob[:, 0 : nor * WO])
            else:
                # ---- main loop: serial layout (f32r) ----
                chunks = [
                    (bi, r0, nxr, oy0, nor)
                    for bi in range(BL)
                    for (r0, nxr, oy0, nor) in CHUNKS
                ]
                with (
                    tc.tile_pool(name="x3", bufs=4) as x3p,
                    tc.tile_pool(name="ps", bufs=8, space="PSUM") as psp,
                    tc.tile_pool(name="ob", bufs=2) as obp,
                ):
                    x3_tiles = {}

                    def issue_x3s(ci):
                        bi, r0, nxr, oy0, nor = chunks[ci]
                        x3 = x3p.tile([96, X3W], stdt, tag="x3", name=f"x3_{ci}")
                        xin = xs[bi, :, r0 : r0 + nxr, :]
                        if mode == "f32r":
                            xin = xin.bitcast(mmdt)
                        nc.sync.dma_start(out=x3[0:32, 0 : nxr * W], in_=xin)
                        nc.scalar.dma_start(
                            out=x3[32:64, 0 : nor * W],
                            in_=x3[0:32, W : (nor + 1) * W],
                        )
                        nc.gpsimd.dma_start(
                            out=x3[64:96, 0 : nor * W],
                            in_=x3[0:32, 2 * W : (nor + 2) * W],
                        )
                        x3_tiles[ci] = x3

                    issue_x3s(0)
                    issue_x3s(1)

                    def issue_bias_s(ci):
                        _, _, _, oy0, nor = chunks[ci]
                        a, b = oy0 * WO, (oy0 + nor) * WO
                        nc.gpsimd.dma_start(out=bias2[0:64, a:b], in_=bias[:, a:b])

                    for ci, (bi, r0, nxr, oy0, nor) in enumerate(chunks):
                        if ci < len(CHUNKS):
                            issue_bias_s(ci)
                        if ci + 2 < len(chunks):
                            issue_x3s(ci + 2)
                        x3 = x3_tiles.pop(ci)
                        x3v = x3.rearrange("p (r c) -> p r c", c=W)
                        # serial layout: 4-row tiles, dx-outer over groups
                        # of 8 tiles (weights change every 8 matmuls)
                        ntiles = (nor + 3) // 4
                        t0 = 0
                        while t0 < ntiles:
                            g = min(8, ntiles - t0)
                            tss = []
                            for ti in range(t0, t0 + g):
                                nr = min(4, nor - 4 * ti)
                                ps = psp.tile([64, 4 * WO], f32, tag="ps")
                                tss.append((ti, nr, ps))
                            for dx in range(3):
                                lw = lhsT[:, dx * COUT : (dx + 1) * COUT]
                                for (ti, nr, ps) in tss:
                                    yl = 4 * ti
                                    nc.tensor.matmul(
                                        ps[:, 0 : nr * WO],
                                        lhsT=lw,
                                        rhs=x3v[:, yl : yl + nr, dx : dx + WO],
                                        start=(dx == 0),
                                        stop=(dx == 2),
                                        skip_group_check=True,
                                    )
                            ob = obp.tile([64, 8 * 4 * WO], f32, tag="ob")
                            rows = 0
                            for gi, (ti, nr, ps) in enumerate(tss):
                                yg = oy0 + 4 * ti
                                N = nr * WO
                                o0 = gi * 4 * WO
                                nc.vector.tensor_add(
                                    out=ob[:, o0 : o0 + N],
                                    in0=ps[:, 0:N],
                                    in1=bias2[0:64, yg * WO : yg * WO + N],
                                )
                                rows += nr
                            yg0 = oy0 + 4 * t0
                            if rows == 4 * g:
                                nc.scalar.dma_start(
                                    out=out[bi, :, yg0 * WO : (yg0 + rows) * WO],
                                    in_=ob[:, 0 : rows * WO],
                                )
                            else:
                                # ragged tail: last tile shorter; store
                                # full tiles in one DMA, tail separately
                                nf = rows - tss[-1][1]
                                nc.scalar.dma_start(
                                    out=out[bi, :, yg0 * WO : (yg0 + nf) * WO],
                                    in_=ob[:, 0 : nf * WO],
                                )
                                nc.scalar.dma_start(
                                    out=out[
                                        bi,
                                        :,
                                        (yg0 + nf) * WO : (yg0 + rows) * WO,
                                    ],
                                    in_=ob[
                                        :,
                                        (g - 1) * 4 * WO : (g - 1) * 4 * WO
                                        + tss[-1][1] * WO,
                                    ],
                                )
                            t0 += g

    if split_waits:
        _split_waits(nc, mybir)
    return nc


def _make_inputs(x, weights, bias, connect_nums):
    """Host-side reshapes only (no input-dependent compute)."""
    x = np.ascontiguousarray(np.asarray(x, dtype=np.float32))
    w = np.ascontiguousarray(
        np.asarray(weights, dtype=np.float32).reshape(COUT, MAXCN * 9)
    )
    b = np.ascontiguousarray(np.asarray(bias, dtype=np.float32).reshape(COUT, HO * WO))
    cnv = np.ascontiguousarray(
        np.asarray(connect_nums, dtype=np.int32).reshape(COUT, 1)
    )
    iota9 = np.ascontiguousarray(
        np.tile(
            np.repeat(np.arange(MAXCN, dtype=np.float32), 9), (COUT, 1)
        )
    )
    ident = np.eye(COUT, dtype=np.float32)
    shards = x.reshape(NCORES, BL, CIN, H, W)
    in_maps = [
        {
            "xs": shards[c],
            "wt": w,
            "bias": b,
            "cn": cnv,
            "iota9": iota9,
            "ident": ident,
        }
        for c in range(NCORES)
    ]
    return in_maps


def _get_runner(mode=_MODE):
    """Build + jit once; reuse across kernel() calls."""
    if mode in _RUNNER_CACHE:
        return _RUNNER_CACHE[mode]

    import jax
    from jax.experimental.shard_map import shard_map
    from jax.sharding import Mesh, PartitionSpec

    import concourse.mybir as mybir
    from concourse.bass2jax import (
        _bass_exec_p,
        install_neuronx_cc_hook,
        partition_id_tensor,
    )

    nc = build_nc(mode)
    install_neuronx_cc_hook()

    partition_name = nc.partition_id_tensor.name if nc.partition_id_tensor else None
    in_names = []
    out_names = []
    out_avals = []
    zero_shapes = []
    for alloc in nc.m.functions[0].allocations:
        if not isinstance(alloc, mybir.MemoryLocationSet):
            continue
        name = alloc.memorylocations[0].name
        if alloc.kind == "ExternalInput":
            if name != partition_name:
                in_names.append(name)
        elif alloc.kind == "ExternalOutput":
            out_names.append(name)
            shape = tuple(alloc.tensor_shape)
            dtype = mybir.dt.np(alloc.dtype)
            out_avals.append(jax.core.ShapedArray(shape, dtype))
            zero_shapes.append((shape, dtype))
    n_params = len(in_names)
    n_outs = len(out_names)
    all_names = in_names + out_names
    if partition_name is not None:
        all_names = all_names + [partition_name]

    def _body(*args):
        operands = list(args)
        if partition_name is not None:
            operands.append(partition_id_tensor())
        outs = _bass_exec_p.bind(
            *operands,
            out_avals=tuple(out_avals),
            in_names=tuple(all_names),
            out_names=tuple(out_names),
            lowering_input_output_aliases=(),
            sim_require_finite=True,
            sim_require_nnan=True,
            nc=nc,
        )
        return tuple(outs)

    devices = jax.devices()[:NCORES]
    mesh = Mesh(np.asarray(devices), ("core",))
    in_specs = (PartitionSpec("core"),) * (n_params + n_outs)
    out_specs = (PartitionSpec("core"),) * n_outs
    sharded = jax.jit(
        shard_map(
            _body, mesh=mesh, in_specs=in_specs, out_specs=out_specs, check_rep=False
        ),
        donate_argnums=tuple(range(n_params, n_params + n_outs)),
        keep_unused=True,
    )

    def run(in_maps):
        concat_in = [
            np.concatenate([np.asarray(in_maps[c][nm]) for c in range(NCORES)], axis=0)
            for nm in in_names
        ]
        concat_zeros = [
            np.zeros((NCORES * s[0],) + tuple(s[1:]), dt) for (s, dt) in zero_shapes
        ]
        out_arrs = sharded(*concat_in, *concat_zeros)
        # bf16 mode stores rounded outputs; widening to f32 is exact
        outv = np.asarray(out_arrs[0]).astype(np.float32)
        return outv.reshape(NCORES, BL, COUT, HO, WO)

    _RUNNER_CACHE[mode] = run
    return run


def kernel(x, weights, bias, connect_nums):
    run = _get_runner()
    in_maps = _make_inputs(x, weights, bias, connect_nums)
    outs = run(in_maps)
    return np.ascontiguousarray(outs.reshape(B, COUT, HO, WO))
